# revision 6
# baseline (speedup 1.0000x reference)
"""GPT2 attention (B=4, S=2048, D=1024, H=16) on 8 trn2 NeuronCores.

Sharding: core = (batch b = cid//2, head-group hg = cid%2 of 8 heads).
Column-parallel c_attn (each core computes q/k/v for its 8 heads),
row-parallel c_proj (each core produces a partial [S, D] output; host
sums the two partials per batch and adds b_proj).

Per-core kernel (all matmuls in float32r = TF32-like, PSUM fp32):
  encT   = enc.T                       (PE transpose via identity)
  qT/kT  = w.T @ encT  (+bias)         [n=64*2heads, t] layout
  v      = encT.T @ wv (+bias)         [f, c] layout, ones col appended
  scores = kT_h.T @ qT_h               [f, t] tile [128, 512]
  exp    = ACT Exp(0.125 * scores)     masked diag tiles via DVE 0/1 mult
  ctx/denom = v_aug.T @ exp            PSUM accumulate over f (causal skip)
  ctxT  /= denom                       (reciprocal + partition_broadcast)
  out_partial = ctxT.T @ w_proj_slice
"""

import os
import sys

sys.path.insert(0, "/opt/trn_rl_repo")

import numpy as np

B, S, D, H = 4, 2048, 1024, 16
HS = D // H            # 64
N_CORES = 8
HPC = H // 2           # heads per core = 8
NW = HPC * HS          # 512 = per-core q/k/v width
TT = 512               # t-tile
FT = 128               # f-tile
NTT = S // TT          # 4
NFT = S // FT          # 16
DC = D // 128          # 8 d-chunks
MASK_NEG = 0.0         # 0/1 multiplicative mask (not additive)

_COMPILED = None


def _build():
    import concourse.bass as bass
    import concourse.tile as tile
    from concourse import bacc, mybir
    from concourse.alu_op_type import AluOpType
    from concourse.masks import make_identity

    f32 = mybir.dt.float32
    f32r = mybir.dt.float32r

    nc = bacc.Bacc("TRN2", target_bir_lowering=False, debug=False,
                   num_devices=N_CORES)

    enc_d = nc.dram_tensor("enc", [S, D], f32, kind="ExternalInput").ap()
    wqk_d = nc.dram_tensor("wqk", [D, 2 * NW], f32, kind="ExternalInput").ap()
    wv_d = nc.dram_tensor("wv", [D, NW], f32, kind="ExternalInput").ap()
    bqk_d = nc.dram_tensor("bqk", [2 * NW, 1], f32, kind="ExternalInput").ap()
    bv_d = nc.dram_tensor("bv", [1, NW], f32, kind="ExternalInput").ap()
    wp_d = nc.dram_tensor("wp", [NW, D], f32, kind="ExternalInput").ap()
    out_d = nc.dram_tensor("out", [S, D], f32, kind="ExternalOutput").ap()

    from contextlib import ExitStack

    with tile.TileContext(nc) as tc, ExitStack() as top:
        # ---- constants ----
        cpool = top.enter_context(tc.tile_pool(name="consts", bufs=1))
        ident = cpool.tile([128, 128], f32r, name="ident")
        masks = [cpool.tile([128, TT], f32r, name=f"mask{k}") for k in range(4)]
        ones_col = cpool.tile([1, 128], f32r, name="ones_col")
        ones8 = cpool.tile([128, 8], f32r, name="ones8")
        with tc.tile_pool(name="cscratch", bufs=2) as csp:
            ident_s = csp.tile([128, 128], f32, name="ident_s", tag="cs")
            make_identity(nc, ident_s[:])
            nc.vector.tensor_copy(ident[:], ident_s[:])
            # 0/1 causal masks for the 4 diagonal offsets: allowed iff
            # global_f <= global_t  <=>  i <= j - 128*k  (f0 = t0 + 128k)
            for k in range(4):
                ms = csp.tile([128, TT], f32, name=f"ms{k}", tag="cs")
                nc.gpsimd.memset(ms[:], 1.0)
                # keep 1.0 where (-i + j - 128k) >= 0, else fill 0.0
                nc.gpsimd.affine_select(
                    out=ms[:], in_=ms[:], compare_op=mybir.AluOpType.is_ge,
                    fill=0.0, base=-128 * k, pattern=[[1, TT]],
                    channel_multiplier=-1)
                nc.vector.tensor_copy(masks[k][:], ms[:])
            ones_s = csp.tile([128, 128], f32, name="ones_s", tag="cs")
            nc.gpsimd.memset(ones_s[:], 1.0)
            nc.vector.tensor_copy(ones_col[:], ones_s[0:1, :])
            nc.vector.tensor_copy(ones8[:], ones_s[:, 0:8])
        bv_sb = cpool.tile([1, NW], f32r, name="bv_sb")
        nc.sync.dma_start(bv_sb[:], bv_d[:].bitcast(f32r))
        bqk_sb = cpool.tile([128, 8], f32, name="bqk_sb")
        nc.sync.dma_start(bqk_sb[:],
                          bqk_d.rearrange("(c p) one -> p (c one)", p=128))
        # persistent stores
        ctxpool = top.enter_context(tc.tile_pool(name="ctxstore", bufs=1))
        ctxT = [ctxpool.tile([128, S], f32r, name=f"ctxT{p}") for p in range(4)]

        # psum pools
        ps512 = top.enter_context(tc.tile_pool(name="ps512", bufs=3, space="PSUM"))
        psctx = top.enter_context(tc.tile_pool(name="psctx", bufs=2, space="PSUM"))

        with ExitStack() as ph13:
            encT_pool = ph13.enter_context(tc.tile_pool(name="encT", bufs=1))
            encT = [encT_pool.tile([128, S], f32r, name=f"encT{j}")
                    for j in range(DC)]
            vpool = ph13.enter_context(tc.tile_pool(name="vstore", bufs=1))
            v_store = [vpool.tile([128, HPC * (HS + 1)], f32r, name=f"v{i}")
                       for i in range(NFT)]
            stage = ph13.enter_context(tc.tile_pool(name="stage", bufs=4))
            wvp = ph13.enter_context(tc.tile_pool(name="wvp", bufs=1))
            wqkp = ph13.enter_context(tc.tile_pool(name="wqkp", bufs=3))
            qkp = ph13.enter_context(tc.tile_pool(name="qkp", bufs=1))
            expp = ph13.enter_context(tc.tile_pool(name="expp", bufs=4))
            rrp = ph13.enter_context(tc.tile_pool(name="rrp", bufs=2))
            rbp = ph13.enter_context(tc.tile_pool(name="rbp", bufs=2))

            # ---- phase 1: encT = enc.T ----
            for j in range(DC):
                for i4 in range(NFT // 4):     # 4 t-blocks of 128 per psum tile
                    pt = ps512.tile([128, 512], f32r, name="pt_tr", tag="ps512")
                    for q in range(4):
                        i = i4 * 4 + q
                        st = stage.tile([128, 128], f32r, name="st", tag="st")
                        nc.sync.dma_start(
                            st[:],
                            enc_d[128 * i:128 * (i + 1),
                                  128 * j:128 * (j + 1)].bitcast(f32r))
                        nc.tensor.transpose(pt[:, 128 * q:128 * (q + 1)],
                                            st[:], ident[:])
                    nc.vector.tensor_copy(
                        encT[j][:, 512 * i4:512 * (i4 + 1)], pt[:])

            # ---- phase 2: v (all 8 heads) ----
            wv_sb = wvp.tile([128, DC, NW], f32r, name="wv_sb")
            nc.sync.dma_start(wv_sb[:],
                              wv_d.rearrange("(c p) n -> p c n", p=128).bitcast(f32r))
            for i in range(NFT):
                ps = ps512.tile([128, NW], f32, name="ps_v", tag="ps512")
                for j in range(DC):
                    nc.tensor.matmul(ps[:], encT[j][:, 128 * i:128 * (i + 1)],
                                     wv_sb[:, j, :], start=(j == 0), stop=False)
                nc.tensor.matmul(ps[:], ones_col[:], bv_sb[:],
                                 start=False, stop=True)
                ones_dst = v_store[i][:].rearrange(
                    "p (h c) -> p h c", c=HS + 1)[:, :, HS:HS + 1]
                nc.vector.tensor_copy(
                    ones_dst, ones8[:].rearrange("p (h o) -> p h o", o=1))
                # strided copy: heads h -> cols [65h, 65h+64)
                dst = v_store[i][:].rearrange("p (h c) -> p h c", c=HS + 1)[:, :, 0:HS]
                src = ps[:].rearrange("p (h c) -> p h c", c=HS)
                nc.vector.tensor_copy(dst, src)

            # ---- phases 3+4 per head-pair ----
            for p in range(4):
                qT = qkp.tile([128, S], f32r, name=f"qT{p}", tag="qT")
                kT = qkp.tile([128, S], f32r, name=f"kT{p}", tag="kT")
                for which, (col0, dst, bcol) in enumerate(
                        [(128 * p, qT, p), (NW + 128 * p, kT, 4 + p)]):
                    wt = wqkp.tile([128, DC, 128], f32r, name="wt", tag="wt")
                    nc.sync.dma_start(
                        wt[:],
                        wqk_d[:, col0:col0 + 128]
                        .rearrange("(c p) n -> p c n", p=128).bitcast(f32r))
                    for ti in range(NTT):
                        ps = ps512.tile([128, TT], f32, name="ps_qk", tag="ps512")
                        for j in range(DC):
                            nc.tensor.matmul(ps[:], wt[:, j, :],
                                             encT[j][:, TT * ti:TT * (ti + 1)],
                                             start=(j == 0), stop=(j == DC - 1))
                        nc.vector.tensor_scalar_add(
                            dst[:, TT * ti:TT * (ti + 1)], ps[:],
                            bqk_sb[:, bcol:bcol + 1])

                for hh in range(2):
                    h = 2 * p + hh
                    q_h = qT[HS * hh:HS * (hh + 1), :]
                    k_h = kT[HS * hh:HS * (hh + 1), :]
                    for ti in range(NTT):
                        t0 = TT * ti
                        nf = 4 * (ti + 1)
                        cps = psctx.tile([HS + 1, TT], f32, name="cps")
                        for fi in range(nf):
                            f0 = FT * fi
                            sc = ps512.tile([128, TT], f32, name="sc", tag="ps512")
                            nc.tensor.matmul(sc[:], k_h[:, f0:f0 + FT],
                                             q_h[:, t0:t0 + TT],
                                             start=True, stop=True)
                            ex = expp.tile([128, TT], f32r, name="ex", tag="ex")
                            nc.scalar.activation(ex[:], sc[:],
                                                 mybir.ActivationFunctionType.Exp,
                                                 scale=1.0 / np.sqrt(HS))
                            kdiag = fi - 4 * ti
                            if kdiag >= 0:
                                nc.vector.tensor_tensor(
                                    ex[:], ex[:], masks[kdiag][:],
                                    op=AluOpType.mult)
                            nc.tensor.matmul(
                                cps[:],
                                v_store[fi][:, (HS + 1) * h:(HS + 1) * (h + 1)],
                                ex[:], start=(fi == 0), stop=(fi == nf - 1))
                        rr = rrp.tile([1, TT], f32, name="rr", tag="rr")
                        nc.vector.reciprocal(rr[:], cps[HS:HS + 1, :])
                        rb = rbp.tile([HS, TT], f32, name="rb", tag="rb")
                        nc.gpsimd.partition_broadcast(rb[:], rr[:])
                        nc.vector.tensor_tensor(
                            ctxT[p][HS * hh:HS * (hh + 1), t0:t0 + TT],
                            cps[0:HS, :], rb[:], op=AluOpType.mult)

        # ---- phase 5: c_proj partial ----
        with ExitStack() as ph5:
            outp = ph5.enter_context(tc.tile_pool(name="outp", bufs=4))
            wpp = ph5.enter_context(tc.tile_pool(name="wpp", bufs=1))
            wp_sb = wpp.tile([128, 4, D], f32r, name="wp_sb")
            nc.sync.dma_start(wp_sb[:],
                              wp_d.rearrange("(c p) n -> p c n", p=128).bitcast(f32r))
            for i in range(NFT):
                for half in range(2):
                    ps = ps512.tile([128, 512], f32, name="ps_o", tag="ps512")
                    for j in range(4):
                        nc.tensor.matmul(ps[:],
                                         ctxT[j][:, 128 * i:128 * (i + 1)],
                                         wp_sb[:, j, 512 * half:512 * (half + 1)],
                                         start=(j == 0), stop=(j == 3))
                    ot = outp.tile([128, 512], f32, name="ot", tag="ot")
                    nc.vector.tensor_copy(ot[:], ps[:])
                    nc.sync.dma_start(
                        out_d[128 * i:128 * (i + 1),
                              512 * half:512 * (half + 1)], ot[:])

    nc.compile()
    return nc


def kernel(encodings, w_attn, b_attn, w_proj, b_proj):
    from concourse.bass_utils import run_bass_kernel_spmd

    global _COMPILED
    if _COMPILED is None:
        _COMPILED = _build()
    nc = _COMPILED

    encodings = np.ascontiguousarray(np.asarray(encodings, dtype=np.float32))
    w_attn = np.asarray(w_attn, dtype=np.float32)
    b_attn = np.asarray(b_attn, dtype=np.float32)
    w_proj = np.asarray(w_proj, dtype=np.float32)
    b_proj = np.asarray(b_proj, dtype=np.float32)

    in_maps = []
    for cid in range(N_CORES):
        b = cid // 2
        hg = cid % 2
        c0 = NW * hg
        wqk = np.concatenate([w_attn[:, c0:c0 + NW],
                              w_attn[:, D + c0:D + c0 + NW]], axis=1)
        bqk = np.concatenate([b_attn[c0:c0 + NW],
                              b_attn[D + c0:D + c0 + NW]])[:, None]
        wv = w_attn[:, 2 * D + c0:2 * D + c0 + NW]
        bv = b_attn[2 * D + c0:2 * D + c0 + NW][None, :]
        wp = w_proj[c0:c0 + NW, :]
        in_maps.append({
            "enc": np.ascontiguousarray(encodings[b]),
            "wqk": np.ascontiguousarray(wqk),
            "wv": np.ascontiguousarray(wv),
            "bqk": np.ascontiguousarray(bqk),
            "bv": np.ascontiguousarray(bv),
            "wp": np.ascontiguousarray(wp),
        })

    trace = bool(os.environ.get("KERNEL_TRACE"))
    kw = {}
    if trace:
        kw.update(trace=True, tmpdir=os.environ.get("KERNEL_TRACE_DIR") or None)
    res = run_bass_kernel_spmd(nc, in_maps, core_ids=list(range(N_CORES)), **kw)
    if trace and res.exec_time_ns is not None:
        print(f"HW exec time: {res.exec_time_ns} ns")

    out = np.empty((B, S, D), dtype=np.float32)
    for b in range(B):
        out[b] = res.results[2 * b]["out"] + res.results[2 * b + 1]["out"] \
            + b_proj[None, :]
    return out


# revision 14
# speedup vs baseline: 1.1352x; 1.1352x over previous
"""GPT2 attention (B=4, S=2048, D=1024, H=16) on 8 trn2 NeuronCores.

Sharding: core = (batch b = cid//2, head-group hg = cid%2 of 8 heads).
Column-parallel c_attn (each core computes q/k/v for its 8 heads),
row-parallel c_proj (each core produces a partial [S, D] output; host
sums the two partials per batch and adds b_proj).

Per-core kernel (all matmuls in float32r = TF32-like, PSUM fp32):
  encT   = enc.T                       (PE transpose via identity)
  qT/kT  = w.T @ encT  (+bias)         [n=64*2heads, t] layout
  v      = encT.T @ wv (+bias)         [f, c] layout, ones col appended
  scores = kT_h.T @ qT_h               [f, t] tile [128, 512]
  exp    = ACT Exp(0.125 * scores)     masked diag tiles via DVE 0/1 mult
  ctx/denom = v_aug.T @ exp            PSUM accumulate over f (causal skip)
  ctxT  /= denom                       (reciprocal + partition_broadcast)
  out_partial = ctxT.T @ w_proj_slice
"""

import os
import sys

sys.path.insert(0, "/opt/trn_rl_repo")

import numpy as np

B, S, D, H = 4, 2048, 1024, 16
HS = D // H            # 64
N_CORES = 8
HPC = H // 2           # heads per core = 8
NW = HPC * HS          # 512 = per-core q/k/v width
TT = 512               # t-tile
FT = 128               # f-tile
NTT = S // TT          # 4
NFT = S // FT          # 16
DC = D // 128          # 8 d-chunks
MASK_NEG = 0.0         # 0/1 multiplicative mask (not additive)

_COMPILED = None


def _build():
    import concourse.bass as bass
    import concourse.tile as tile
    from concourse import bacc, mybir
    from concourse.alu_op_type import AluOpType
    from concourse.masks import make_identity

    f32 = mybir.dt.float32
    f32r = mybir.dt.float32r

    nc = bacc.Bacc("TRN2", target_bir_lowering=False, debug=False,
                   num_devices=N_CORES)

    enc_d = nc.dram_tensor("enc", [S, D], f32, kind="ExternalInput").ap()
    wqk_d = nc.dram_tensor("wqk", [D, 2 * NW], f32, kind="ExternalInput").ap()
    wv_d = nc.dram_tensor("wv", [D, NW], f32, kind="ExternalInput").ap()
    bqk_d = nc.dram_tensor("bqk", [2 * NW, 1], f32, kind="ExternalInput").ap()
    bv_d = nc.dram_tensor("bv", [1, NW], f32, kind="ExternalInput").ap()
    wp_d = nc.dram_tensor("wp", [NW, D], f32, kind="ExternalInput").ap()
    out_d = nc.dram_tensor("out", [S, D], f32, kind="ExternalOutput").ap()

    from contextlib import ExitStack

    with tile.TileContext(nc) as tc, ExitStack() as top:
        # ---- constants ----
        cpool = top.enter_context(tc.tile_pool(name="consts", bufs=1))
        ident = cpool.tile([128, 128], f32r, name="ident")
        masks = [cpool.tile([128, TT], f32r, name=f"mask{k}") for k in range(4)]
        ones_col = cpool.tile([1, 128], f32r, name="ones_col")
        ones8 = cpool.tile([128, 8], f32r, name="ones8")
        with tc.tile_pool(name="cscratch", bufs=2) as csp:
            ident_s = csp.tile([128, 128], f32, name="ident_s", tag="cs")
            make_identity(nc, ident_s[:])
            nc.vector.tensor_copy(ident[:], ident_s[:])
            # 0/1 causal masks for the 4 diagonal offsets: allowed iff
            # global_f <= global_t  <=>  i <= j - 128*k  (f0 = t0 + 128k)
            for k in range(4):
                ms = csp.tile([128, TT], f32, name=f"ms{k}", tag="cs")
                nc.gpsimd.memset(ms[:], 0.0)
                # keep 0.0 where (-i + j - 128k) >= 0, else fill -30000
                nc.gpsimd.affine_select(
                    out=ms[:], in_=ms[:], compare_op=mybir.AluOpType.is_ge,
                    fill=-30000.0, base=-128 * k, pattern=[[1, TT]],
                    channel_multiplier=-1)
                nc.vector.tensor_copy(masks[k][:], ms[:])
            ones_s = csp.tile([128, 128], f32, name="ones_s", tag="cs")
            nc.gpsimd.memset(ones_s[:], 1.0)
            nc.vector.tensor_copy(ones_col[:], ones_s[0:1, :])
            nc.vector.tensor_copy(ones8[:], ones_s[:, 0:8])
        bv_sb = cpool.tile([1, NW], f32r, name="bv_sb")
        nc.sync.dma_start(bv_sb[:], bv_d[:].bitcast(f32r))
        bqk_sb = cpool.tile([128, 8], f32, name="bqk_sb")
        nc.sync.dma_start(bqk_sb[:],
                          bqk_d.rearrange("(c p) one -> p (c one)", p=128))
        # persistent stores
        ctxpool = top.enter_context(tc.tile_pool(name="ctxstore", bufs=1))
        ctxT = [ctxpool.tile([128, S], f32r, name=f"ctxT{p}") for p in range(4)]

        # psum pools
        ps512 = top.enter_context(tc.tile_pool(name="ps512", bufs=4, space="PSUM"))
        psctx = top.enter_context(tc.tile_pool(name="psctx", bufs=2, space="PSUM"))

        with ExitStack() as ph13:
            encT_pool = ph13.enter_context(tc.tile_pool(name="encT", bufs=1))
            encT = [encT_pool.tile([128, S], f32r, name=f"encT{j}")
                    for j in range(DC)]
            vpool = ph13.enter_context(tc.tile_pool(name="vstore", bufs=1))
            v_store = [vpool.tile([128, HPC * (HS + 1)], f32r, name=f"v{i}")
                       for i in range(NFT)]
            stage = ph13.enter_context(tc.tile_pool(name="stage", bufs=3))
            wvp = ph13.enter_context(tc.tile_pool(name="wvp", bufs=1))
            wqkp = ph13.enter_context(tc.tile_pool(name="wqkp", bufs=2))
            qkp = ph13.enter_context(tc.tile_pool(name="qkp", bufs=1))
            expp = ph13.enter_context(tc.tile_pool(name="expp", bufs=5))
            rrp = ph13.enter_context(tc.tile_pool(name="rrp", bufs=1))
            rbp = ph13.enter_context(tc.tile_pool(name="rbp", bufs=2))

            # ---- phase 1: encT = enc.T ----
            for j in range(DC):
                for i4 in range(NFT // 4):     # 4 t-blocks of 128 per psum tile
                    pt = ps512.tile([128, 512], f32r, name="pt_tr", tag="ps512")
                    for q in range(4):
                        i = i4 * 4 + q
                        st = stage.tile([128, 128], f32r, name="st", tag="st")
                        nc.sync.dma_start(
                            st[:],
                            enc_d[128 * i:128 * (i + 1),
                                  128 * j:128 * (j + 1)].bitcast(f32r))
                        nc.tensor.transpose(pt[:, 128 * q:128 * (q + 1)],
                                            st[:], ident[:])
                    nc.vector.tensor_copy(
                        encT[j][:, 512 * i4:512 * (i4 + 1)], pt[:])

            # ---- phase 2: v (all 8 heads) ----
            wv_sb = wvp.tile([128, DC, NW], f32r, name="wv_sb")
            nc.sync.dma_start(wv_sb[:],
                              wv_d.rearrange("(c p) n -> p c n", p=128).bitcast(f32r))
            for i in range(NFT):
                ps = ps512.tile([128, NW], f32, name="ps_v", tag="ps512")
                for j in range(DC):
                    nc.tensor.matmul(ps[:], encT[j][:, 128 * i:128 * (i + 1)],
                                     wv_sb[:, j, :], start=(j == 0), stop=False)
                nc.tensor.matmul(ps[:], ones_col[:], bv_sb[:],
                                 start=False, stop=True)
                ones_dst = v_store[i][:].rearrange(
                    "p (h c) -> p h c", c=HS + 1)[:, :, HS:HS + 1]
                nc.vector.tensor_copy(
                    ones_dst, ones8[:].rearrange("p (h o) -> p h o", o=1))
                # strided copy: heads h -> cols [65h, 65h+64)
                dst = v_store[i][:].rearrange("p (h c) -> p h c", c=HS + 1)[:, :, 0:HS]
                src = ps[:].rearrange("p (h c) -> p h c", c=HS)
                nc.vector.tensor_copy(dst, src)

            # ---- phases 3+4 per head-pair ----
            for p in range(4):
                qT = qkp.tile([128, S], f32r, name=f"qT{p}", tag="qT")
                kT = qkp.tile([128, S], f32r, name=f"kT{p}", tag="kT")
                for which, (col0, dst, bcol) in enumerate(
                        [(128 * p, qT, p), (NW + 128 * p, kT, 4 + p)]):
                    wt = wqkp.tile([128, DC, 128], f32r, name="wt", tag="wt")
                    nc.sync.dma_start(
                        wt[:],
                        wqk_d[:, col0:col0 + 128]
                        .rearrange("(c p) n -> p c n", p=128).bitcast(f32r))
                    for ti in range(NTT):
                        ps = ps512.tile([128, TT], f32, name="ps_qk", tag="ps512")
                        for j in range(DC):
                            nc.tensor.matmul(ps[:], wt[:, j, :],
                                             encT[j][:, TT * ti:TT * (ti + 1)],
                                             start=(j == 0), stop=(j == DC - 1))
                        nc.vector.tensor_scalar_add(
                            dst[:, TT * ti:TT * (ti + 1)], ps[:],
                            bqk_sb[:, bcol:bcol + 1])

                # software-pipelined attention for the two heads of this
                # pair: emit scores/exp L tiles ahead of the ctx matmuls so
                # the PE never waits on the ACT exp chain.
                LOOKAHEAD = 3
                denoms = rrp.tile([8, TT], f32, name=f"denoms{p}", tag="rr", bufs=2)
                tasks = []   # (h-in-pair, ti, nf)
                for hh in range(2):
                    for ti in range(NTT):
                        tasks.append((hh, ti, 4 * (ti + 1)))

                def emit_scores(hh, ti, fi, p=p, qT=qT, kT=kT):
                    q_h = qT[HS * hh:HS * (hh + 1), :]
                    k_h = kT[HS * hh:HS * (hh + 1), :]
                    t0 = TT * ti
                    f0 = FT * fi
                    sc = ps512.tile([128, TT], f32, name="sc", tag="ps512")
                    kdiag = fi - 4 * ti
                    nc.tensor.matmul(sc[:], k_h[:, f0:f0 + FT],
                                     q_h[:, t0:t0 + TT],
                                     start=True, stop=(kdiag < 0))
                    if kdiag >= 0:
                        nc.tensor.matmul(sc[:], ident[:], masks[kdiag][:],
                                         start=False, stop=True)
                    ex = expp.tile([128, TT], f32r, name="ex", tag="ex")
                    nc.scalar.activation(ex[:], sc[:],
                                         mybir.ActivationFunctionType.Exp,
                                         scale=1.0 / np.sqrt(HS))
                    return ex

                for hh, ti, nf in tasks:
                    h = 2 * p + hh
                    t0 = TT * ti
                    cps = psctx.tile([HS + 1, TT], f32, name="cps")
                    pend = []
                    for fi in range(min(LOOKAHEAD, nf)):
                        pend.append(emit_scores(hh, ti, fi))
                    for fi in range(nf):
                        if fi + LOOKAHEAD < nf:
                            pend.append(emit_scores(hh, ti, fi + LOOKAHEAD))
                        ex = pend.pop(0)
                        nc.tensor.matmul(
                            cps[:],
                            v_store[fi][:, (HS + 1) * h:(HS + 1) * (h + 1)],
                            ex[:], start=(fi == 0), stop=(fi == nf - 1))
                    # denom row out via DMA (frees psum fast); body copy
                    drow = 4 * hh + ti
                    trow = rbp.tile([1, TT], f32, name="trow", tag="trow")
                    nc.vector.tensor_copy(trow[:], cps[HS:HS + 1, :])
                    nc.sync.dma_start(denoms[drow:drow + 1, :], trow[:])
                    nc.vector.tensor_copy(
                        ctxT[p][HS * hh:HS * (hh + 1), t0:t0 + TT],
                        cps[0:HS, :])
                # batched reciprocal for the whole pair, then in-place norm
                rsc = rrp.tile([8, TT], f32, name=f"rsc{p}", tag="rsc")
                rcp = rrp.tile([8, TT], f32, name=f"rcp{p}", tag="rcp")
                nc.vector.reciprocal_approx_accurate(rcp[:], denoms[:], rsc[:])
                for hh, ti, nf in tasks:
                    drow = 4 * hh + ti
                    rrow = rbp.tile([1, TT], f32, name="rrow", tag="trow")
                    nc.sync.dma_start(rrow[:], rcp[drow:drow + 1, :])
                    rb = rbp.tile([128, TT], f32, name="rb", tag="rb")
                    nc.gpsimd.partition_broadcast(rb[:], rrow[:])
                    dst = ctxT[p][HS * hh:HS * (hh + 1),
                                  TT * ti:TT * (ti + 1)]
                    nc.vector.tensor_tensor(
                        dst, dst, rb[HS * hh:HS * (hh + 1), :],
                        op=AluOpType.mult)

        # ---- phase 5: c_proj partial ----
        with ExitStack() as ph5:
            outp = ph5.enter_context(tc.tile_pool(name="outp", bufs=4))
            wpp = ph5.enter_context(tc.tile_pool(name="wpp", bufs=1))
            wp_sb = wpp.tile([128, 4, D], f32r, name="wp_sb")
            nc.sync.dma_start(wp_sb[:],
                              wp_d.rearrange("(c p) n -> p c n", p=128).bitcast(f32r))
            for i in range(NFT):
                for half in range(2):
                    ps = ps512.tile([128, 512], f32, name="ps_o", tag="ps512")
                    for j in range(4):
                        nc.tensor.matmul(ps[:],
                                         ctxT[j][:, 128 * i:128 * (i + 1)],
                                         wp_sb[:, j, 512 * half:512 * (half + 1)],
                                         start=(j == 0), stop=(j == 3))
                    ot = outp.tile([128, 512], f32, name="ot", tag="ot")
                    nc.vector.tensor_copy(ot[:], ps[:])
                    nc.sync.dma_start(
                        out_d[128 * i:128 * (i + 1),
                              512 * half:512 * (half + 1)], ot[:])

    nc.compile()
    return nc


def kernel(encodings, w_attn, b_attn, w_proj, b_proj):
    from concourse.bass_utils import run_bass_kernel_spmd

    global _COMPILED
    if _COMPILED is None:
        _COMPILED = _build()
    nc = _COMPILED

    encodings = np.ascontiguousarray(np.asarray(encodings, dtype=np.float32))
    w_attn = np.asarray(w_attn, dtype=np.float32)
    b_attn = np.asarray(b_attn, dtype=np.float32)
    w_proj = np.asarray(w_proj, dtype=np.float32)
    b_proj = np.asarray(b_proj, dtype=np.float32)

    in_maps = []
    for cid in range(N_CORES):
        b = cid // 2
        hg = cid % 2
        c0 = NW * hg
        wqk = np.concatenate([w_attn[:, c0:c0 + NW],
                              w_attn[:, D + c0:D + c0 + NW]], axis=1)
        bqk = np.concatenate([b_attn[c0:c0 + NW],
                              b_attn[D + c0:D + c0 + NW]])[:, None]
        wv = w_attn[:, 2 * D + c0:2 * D + c0 + NW]
        bv = b_attn[2 * D + c0:2 * D + c0 + NW][None, :]
        wp = w_proj[c0:c0 + NW, :]
        in_maps.append({
            "enc": np.ascontiguousarray(encodings[b]),
            "wqk": np.ascontiguousarray(wqk),
            "wv": np.ascontiguousarray(wv),
            "bqk": np.ascontiguousarray(bqk),
            "bv": np.ascontiguousarray(bv),
            "wp": np.ascontiguousarray(wp),
        })

    trace = bool(os.environ.get("KERNEL_TRACE"))
    kw = {}
    if trace:
        kw.update(trace=True, tmpdir=os.environ.get("KERNEL_TRACE_DIR") or None)
    res = run_bass_kernel_spmd(nc, in_maps, core_ids=list(range(N_CORES)), **kw)
    if trace and res.exec_time_ns is not None:
        print(f"HW exec time: {res.exec_time_ns} ns")

    out = np.empty((B, S, D), dtype=np.float32)
    for b in range(B):
        out[b] = res.results[2 * b]["out"] + res.results[2 * b + 1]["out"] \
            + b_proj[None, :]
    return out
